# revision 37
# baseline (speedup 1.0000x reference)
"""Trainium2 Bass kernel for nn_AlterCoAttn (alternating co-attention).

Math notes
----------
In the reference, attn1 and attn3 run `_attn` on X = ques_feat[:, None, :]
(sequence length 1).  Softmax over a single element is identically 1.0, so
both are exact no-ops: ques_self == ques_feat and ques_attn == ques_feat
(bitwise, since the output is 1.0 * X).  The only real compute is attn2:

    g      = ques_feat @ w2g + b2g + b2x                       (B, H)   tiny
    featT  = w2x.T @ img[b].T  (+ g[b] per-H bias)             per batch
    h      = tanh(featT)                                       (H, S)
    scoreT = h.T @ w2h                                         (S, 1)
    a      = softmax(scoreT)          (b2h is softmax-invariant)
    out[b] = img[b].T @ a                                      (I,)

Strategy: pure data parallel over batch (64 batches/core x 8 cores).  The
tiny guidance GEMM g is computed on host.  img_feat is shipped once, in
bf16, pre-transposed on host (TensorE contracts over partitions, so I must
sit on partitions for the feat matmul).  The final weighted sum over S is
done on VectorE over the same transposed tiles (bf16 multiply at 2x mode,
a fold-in-half add, then a 1x free-dim reduce), with the softmax row
replicated across partitions by gpsimd.partition_broadcast.  The last two
pairs route their weighted sum through the by-then-idle TensorE/ScalarE
instead (PE transpose of e + (M=1,N=512) matmuls over a small natural-
layout copy) to trim the end-of-kernel VectorE tail.  All matmuls run
bf16 with fp32 PSUM accumulation; measured ~407us HW exec at the full
2.4GHz PE clock (~485us when the chip is P0 power-throttled to 2.0GHz),
PE >99% dense between the fixed ~7us NEFF preamble and ~12us drain
barrier; img_attn rel err ~3.8e-3, ques_attn bit-exact.

Device layouts (per core, P=128):
  imgt [pair, p, k, b2, s] = img[2*pair+b2, s, 128*k+p]   bf16  (feat rhs)
  w2xp [p, k, h]           = w2x[128*k+p, h]              bf16  (feat lhsT)
  w2hp [p, c]              = w2h[128*c+p]                 bf16
  gt   [p, c, b]           = g[b, 128*c+p]                f32   (tanh bias)
  oattn[b, p, n]           = img_attn[b, 128*n+p]         f32   (output)
"""

import numpy as np
import ml_dtypes

BF16 = ml_dtypes.bfloat16

B, S, I, Q, H = 512, 196, 2048, 1024, 512
NCORES = 8
BPC = B // NCORES          # 64 batches per core
P = 128
KI = I // P                # 16 contraction tiles over I
CH = H // P                # 4 chunks over H
NI = I // P                # 16 output column tiles

_CACHE = {}


def build_bass(npairs, fp8=False):
    """Build the per-core Bass program processing 2*npairs batches."""
    import concourse.tile as tile
    from concourse import bacc, mybir
    from concourse.bass import ts

    f32 = mybir.dt.float32
    bf16 = mybir.dt.bfloat16
    fp8e4 = mybir.dt.float8e4
    Act = mybir.ActivationFunctionType
    nb = 2 * npairs

    nc = bacc.Bacc("TRN2", target_bir_lowering=False, debug=False)

    # the last LP pairs finish their weighted sum on TensorE/ScalarE (which
    # are idle by then) instead of VectorE, trimming the end-of-kernel DVE
    # tail; they need a small natural-layout copy of those batches
    LP = 2 if npairs > 4 else 0

    imgt = nc.dram_tensor("imgt", [npairs, P, KI, 2 * S], bf16, kind="ExternalInput").ap()
    w2xp = nc.dram_tensor("w2xp", [P, KI, H], bf16, kind="ExternalInput").ap()
    w2hp = nc.dram_tensor("w2hp", [P, CH], bf16, kind="ExternalInput").ap()
    gt = nc.dram_tensor("gt", [P, CH, nb], f32, kind="ExternalInput").ap()
    oattn = nc.dram_tensor("oattn", [nb, P, NI], f32, kind="ExternalOutput").ap()
    if LP:
        natl = nc.dram_tensor("natl", [2 * LP, S, I], bf16, kind="ExternalInput").ap()
        oattn_l = nc.dram_tensor("oattn_l", [2 * LP, I], f32, kind="ExternalOutput").ap()
    if fp8:
        imgt8 = nc.dram_tensor(
            "imgt8", [npairs, P, KI, 2 * S], fp8e4, kind="ExternalInput"
        ).ap()
        w2xp8 = nc.dram_tensor("w2xp8", [P, KI, H], fp8e4, kind="ExternalInput").ap()

    with tile.TileContext(nc) as tc:
        with (
            tc.tile_pool(name="const", bufs=1) as const,
            tc.tile_pool(name="sb", bufs=5) as sb,
            tc.tile_pool(name="sbs", bufs=3) as sbs,
            tc.tile_pool(name="sbo", bufs=1) as sbo,
            tc.tile_pool(name="psf", bufs=1, space="PSUM") as psf,
            tc.tile_pool(name="pss", bufs=4, space="PSUM") as pss,
        ):
            if fp8:
                w2x_sb = const.tile([P, KI, H], fp8e4)
                w2x_src = w2xp8
            else:
                w2x_sb = const.tile([P, KI, H], bf16)
                w2x_src = w2xp
            # split and interleave the startup-critical loads so the first
            # feat matmuls can begin as soon as their k-slices land
            imgt0_t = sb.tile([P, KI, 2 * S], bf16, tag="imgt")
            w2h_sb = const.tile([P, CH], bf16)
            gt_sb = const.tile([P, CH, nb], f32)
            # pair-0 img rides the scalar HWDGE queue, in parallel with the
            # weights on the sync queue (ScalarE has no other work yet)
            for k2 in range(0, KI, 2):
                nc.sync.dma_start(w2x_sb[:, k2 : k2 + 2], w2x_src[:, k2 : k2 + 2])
                nc.scalar.dma_start(imgt0_t[:, k2 : k2 + 2], imgt[0, :, k2 : k2 + 2])
                if k2 == 0:
                    nc.sync.dma_start(w2h_sb[:], w2hp[:])
                    nc.sync.dma_start(gt_sb[:], gt[:])
            if LP:
                natl_sb = []
                for li in range(2 * LP):
                    na_l = const.tile([P, I], bf16, tag=f"natla{li}")
                    nb_l = const.tile([P, I], bf16, tag=f"natlb{li}")
                    natl_sb.append((na_l, nb_l))
                id11 = const.tile([1, 1], bf16)
                nc.vector.memset(id11[:], 1.0)
            for pair in range(npairs):
                if pair == 0:
                    imgt_t = imgt0_t
                else:
                    imgt_t = sb.tile([P, KI, 2 * S], bf16, tag="imgt")
                    nc.sync.dma_start(imgt_t[:], imgt[pair])
                if LP and pair == npairs - LP - 2:
                    # prefetch the natural-layout tail batches (low priority:
                    # emitted only now, two pairs before they are consumed)
                    for li in range(2 * LP):
                        na_l, nb_l = natl_sb[li]
                        nc.sync.dma_start(na_l[:], natl[li, 0:P, :])
                        nc.sync.dma_start(nb_l[0 : S - P, :], natl[li, P:S, :])
                if fp8:
                    imgt8_t = sb.tile([P, KI, 2 * S], fp8e4, tag="imgt8")
                    nc.sync.dma_start(imgt8_t[:], imgt8[pair])
                    feat_rhs = imgt8_t
                else:
                    feat_rhs = imgt_t

                # featT: (H-chunk on partitions, 2*S free), accumulate over I.
                # One PSUM bank per H-chunk so the next pair's chunk-c matmuls
                # only wait on this pair's chunk-c tanh.  fp8 uses DoubleRow
                # (two I-tiles per matmul); w2x is pre-scaled by 32 on host to
                # stay out of e4m3 subnormals and the tanh scale undoes it.
                fpcs = []
                for c in range(CH):
                    fps = psf.tile([P, 512], f32, tag=f"feat{c}")
                    if fp8:
                        for k in range(0, KI, 2):
                            nc.tensor.matmul(
                                fps[:, 0 : 2 * S],
                                w2x_sb[:, k : k + 2, ts(c, P)],
                                feat_rhs[:, k : k + 2, :],
                                start=(k == 0),
                                stop=(k == KI - 2),
                                perf_mode=mybir.MatmulPerfMode.DoubleRow,
                            )
                    else:
                        for k in range(KI):
                            nc.tensor.matmul(
                                fps[:, 0 : 2 * S],
                                w2x_sb[:, k, ts(c, P)],
                                feat_rhs[:, k],
                                start=(k == 0),
                                stop=(k == KI - 1),
                            )
                    fpcs.append(fps)

                # h = tanh(featT + g[b])  (g is a per-partition bias)
                h_t = sbs.tile([P, CH, 2, S], bf16, tag="ht")
                for c in range(CH):
                    for b2 in range(2):
                        bidx = 2 * pair + b2
                        nc.scalar.activation(
                            h_t[:, c, b2],
                            fpcs[c][:, b2 * S : (b2 + 1) * S],
                            Act.Tanh,
                            bias=gt_sb[:, c, bidx : bidx + 1],
                            scale=(1.0 / 32.0) if fp8 else 1.0,
                        )

                # score (1, 2*S) = w2h.T @ h for both batches at once
                sc_ps = pss.tile([1, 2 * S], f32, tag="sm")
                for c in range(CH):
                    nc.tensor.matmul(
                        sc_ps[:, :],
                        w2h_sb[:, c : c + 1],
                        h_t[:, c],
                        start=(c == 0),
                        stop=(c == CH - 1),
                    )

                # e = exp(score) (scores are O(1): no max-subtraction) with the
                # softmax denominator accumulated for free
                e_row = sbs.tile([1, 2, S], bf16, tag="er")
                sums = sbs.tile([1, 2], f32, tag="sums")
                rc = sbs.tile([1, 2], f32, tag="rc")
                for b2 in range(2):
                    nc.scalar.activation(
                        e_row[:, b2],
                        sc_ps[:, b2 * S : (b2 + 1) * S],
                        Act.Exp,
                        accum_out=sums[:, b2 : b2 + 1],
                    )
                    nc.vector.reciprocal(rc[:, b2 : b2 + 1], sums[:, b2 : b2 + 1])

                SH = S // 2
                tail_pe = LP and pair >= npairs - LP
                for b2 in range(2):
                    bidx = 2 * pair + b2

                    if tail_pe:
                        # transpose e to (S on partitions, 1), weighted sum as
                        # (M=1, N=512) matmuls over the natural-layout copy,
                        # normalization riding the PSUM->SBUF copy on ScalarE
                        li = 2 * (pair - (npairs - LP)) + b2
                        na_l, nb_l = natl_sb[li]
                        et_a = pss.tile([P, 1], bf16, tag="sm")
                        nc.tensor.transpose(et_a[:, :], e_row[:, b2, 0:P], id11[:, :])
                        et_b = pss.tile([P, 1], bf16, tag="sm")
                        nc.tensor.transpose(et_b[0 : S - P, :], e_row[:, b2, P:S], id11[:, :])
                        eta = sbs.tile([P, 1], bf16, tag="eta")
                        nc.vector.tensor_copy(eta[:, :], et_a[:, :])
                        etb = sbs.tile([P, 1], bf16, tag="etb")
                        nc.vector.tensor_copy(etb[0 : S - P, :], et_b[0 : S - P, :])
                        orow = sbo.tile([1, I], f32, tag=f"or{b2}")
                        for n in range(4):
                            o_ps = pss.tile([1, 512], f32, tag="sm")
                            nc.tensor.matmul(
                                o_ps[:, :], eta[:, :], na_l[:, ts(n, 512)],
                                start=True, stop=False,
                            )
                            nc.tensor.matmul(
                                o_ps[:, :], etb[0 : S - P, :], nb_l[0 : S - P, ts(n, 512)],
                                start=False, stop=True,
                            )
                            nc.scalar.mul(orow[:, ts(n, 512)], o_ps[:, :], rc[:, b2 : b2 + 1])
                        nc.sync.dma_start(oattn_l[li : li + 1, :], orow[:, :])
                        continue

                    # a = e / sum(e), then replicate the row to all 128
                    # partitions on the GpSimd engine
                    a_row = sbs.tile([1, S], bf16, tag=f"ar{b2}")
                    nc.vector.tensor_scalar_mul(
                        a_row[:, :], e_row[:, b2], rc[:, b2 : b2 + 1]
                    )
                    a_bc = sbs.tile([P, 1, S], bf16, tag=f"abc{b2}")
                    nc.gpsimd.partition_broadcast(a_bc[:, 0, :], a_row[:, :])

                    # img_attn[b, 128n+p] = sum_s imgT[p, n, s] * a[s]:
                    # bf16 multiply, fold S in half with one add, then a
                    # free-dim reduce -- reusing the transposed feat tiles
                    scratch = sbs.tile([P, KI, S], bf16, tag=f"sc{b2}")
                    nc.vector.tensor_tensor(
                        scratch[:],
                        imgt_t[:, :, b2 * S : (b2 + 1) * S],
                        a_bc[:].to_broadcast([P, KI, S]),
                        op=mybir.AluOpType.mult,
                    )
                    fold = sbs.tile([P, KI, SH], bf16, tag=f"fo{b2}")
                    nc.vector.tensor_tensor(
                        fold[:],
                        scratch[:, :, 0:SH],
                        scratch[:, :, SH:S],
                        op=mybir.AluOpType.add,
                    )
                    obt = sbs.tile([P, NI], f32, tag=f"obt{b2}")
                    nc.vector.tensor_reduce(
                        obt[:, :], fold[:], axis=mybir.AxisListType.X,
                        op=mybir.AluOpType.add,
                    )
                    nc.sync.dma_start(oattn[bidx], obt[:, :])

    nc.compile()
    return nc


def _bf16(x):
    return np.asarray(x, dtype=np.float32).astype(BF16)


def _fp8np():
    from concourse import mybir

    return mybir.dt.np(mybir.dt.float8e4)


def prep_core_inputs(img_core, g_core, fp8=False):
    """Host-side layout prep for one core's 64 batches."""
    nb = img_core.shape[0]
    npairs = nb // 2
    img_bf = _bf16(img_core)                                   # (nb, S, I)
    # [pair, p, k, b2, s] = img[2*pair+b2, s, 128*k+p]
    imgt = np.ascontiguousarray(
        img_bf.reshape(npairs, 2, S, KI, P).transpose(0, 4, 3, 1, 2)
    ).reshape(npairs, P, KI, 2 * S)
    # [p, c, b] = g[b, 128*c+p]
    gt = np.ascontiguousarray(
        np.asarray(g_core, dtype=np.float32).reshape(nb, CH, P).transpose(2, 1, 0)
    )
    m = {"imgt": imgt, "gt": gt}
    LP = 2 if npairs > 4 else 0
    if LP:
        m["natl"] = np.ascontiguousarray(img_bf[-2 * LP :])
    if fp8:
        m["imgt8"] = imgt.astype(_fp8np())
    return m


def prep_weight_inputs(w2x, w2h, fp8=False):
    w2xp = np.ascontiguousarray(_bf16(w2x).reshape(KI, P, H).transpose(1, 0, 2))
    w2hp = np.ascontiguousarray(_bf16(w2h).reshape(CH, P).T)
    m = {"w2xp": w2xp, "w2hp": w2hp}
    if fp8:
        m["w2xp8"] = np.ascontiguousarray(
            (np.asarray(w2x, np.float32) * 32.0).reshape(KI, P, H).transpose(1, 0, 2)
        ).astype(_fp8np())
    return m


def run_device(img_feat, g, wm, trace=False, fp8=False):
    """Run the 8-core SPMD kernel; returns (img_attn (B, I) f32, results obj)."""
    from concourse.bass_utils import run_bass_kernel_spmd

    key = (BPC // 2, fp8)
    if key not in _CACHE:
        _CACHE[key] = build_bass(BPC // 2, fp8=fp8)
    nc = _CACHE[key]

    in_maps = []
    for c in range(NCORES):
        sl = slice(c * BPC, (c + 1) * BPC)
        m = prep_core_inputs(img_feat[sl], g[sl], fp8=fp8)
        m.update(wm)
        in_maps.append(m)
    res = run_bass_kernel_spmd(nc, in_maps, list(range(NCORES)), trace=trace)
    parts = []
    for r in res.results:
        core_out = r["oattn"].transpose(0, 2, 1).reshape(BPC, I)
        if "oattn_l" in r:
            nl = r["oattn_l"].shape[0]
            core_out = np.concatenate([core_out[: BPC - nl], r["oattn_l"]], axis=0)
        parts.append(core_out)
    return np.concatenate(parts, axis=0), res


def kernel(
    ques_feat,
    img_feat,
    w1x,
    b1x,
    w1h,
    b1h,
    w2x,
    b2x,
    w2g,
    b2g,
    w2h,
    b2h,
    w3x,
    b3x,
    w3g,
    b3g,
    w3h,
    b3h,
    trace=False,
    fp8=False,
):
    ques_feat = np.asarray(ques_feat, dtype=np.float32)
    img_feat = np.asarray(img_feat)

    # attn1/attn3 are exact no-ops (softmax over a length-1 axis == 1.0)
    ques_attn = ques_feat.copy()

    # tiny guidance GEMM on host (b2h shifts scores uniformly -> softmax-invariant)
    g = ques_feat @ np.asarray(w2g, dtype=np.float32)
    g = g + np.asarray(b2g, dtype=np.float32) + np.asarray(b2x, dtype=np.float32)

    wm = prep_weight_inputs(w2x, w2h, fp8=fp8)
    img_attn, _ = run_device(img_feat, g, wm, trace=trace, fp8=fp8)
    return (ques_attn, img_attn)


if __name__ == "__main__":
    rng = np.random.default_rng(0)
    ins = {
        "ques_feat": rng.standard_normal((B, Q), dtype=np.float32),
        "img_feat": rng.standard_normal((B, S, I), dtype=np.float32),
        "w2x": rng.standard_normal((I, H), dtype=np.float32) * (I ** -0.5),
        "b2x": np.zeros(H, np.float32),
        "w2g": rng.standard_normal((Q, H), dtype=np.float32) * (Q ** -0.5),
        "b2g": np.zeros(H, np.float32),
        "w2h": rng.standard_normal(H, dtype=np.float32) * (H ** -0.5),
        "b2h": np.float32(0.0),
    }
    zeros = {k: np.float32(0.0) for k in
             ["w1x", "b1x", "w1h", "b1h", "w3x", "b3x", "w3g", "b3g", "w3h", "b3h"]}
    out = kernel(**ins, **zeros)
    print(out[0].shape, out[1].shape)


# revision 39
# speedup vs baseline: 1.2029x; 1.2029x over previous
"""Trainium2 Bass kernel for nn_AlterCoAttn (alternating co-attention).

Math notes
----------
In the reference, attn1 and attn3 run `_attn` on X = ques_feat[:, None, :]
(sequence length 1).  Softmax over a single element is identically 1.0, so
both are exact no-ops: ques_self == ques_feat and ques_attn == ques_feat
(bitwise, since the output is 1.0 * X).  The only real compute is attn2:

    g      = ques_feat @ w2g + b2g + b2x                       (B, H)   tiny
    featT  = w2x.T @ img[b].T  (+ g[b] per-H bias)             per batch
    h      = tanh(featT)                                       (H, S)
    scoreT = h.T @ w2h                                         (S, 1)
    a      = softmax(scoreT)          (b2h is softmax-invariant)
    out[b] = img[b].T @ a                                      (I,)

Strategy: pure data parallel over batch (64 batches/core x 8 cores).  The
tiny guidance GEMM g is computed on host.  img_feat is shipped once, in
bf16, pre-transposed on host (TensorE contracts over partitions, so I must
sit on partitions for the feat matmul).  The final weighted sum over S is
done on VectorE over the same transposed tiles (bf16 multiply at 2x mode,
a fold-in-half add, then a 1x free-dim reduce), with the softmax row
replicated across partitions by gpsimd.partition_broadcast.  The last two
pairs route their weighted sum through the by-then-idle TensorE/ScalarE
instead (PE transpose of e + (M=1,N=512) matmuls over a small natural-
layout copy) to trim the end-of-kernel VectorE tail.  All matmuls run
bf16 with fp32 PSUM accumulation; measured ~407us HW exec at the full
2.4GHz PE clock (~485us when the chip is P0 power-throttled to 2.0GHz),
PE >99% dense between the fixed ~7us NEFF preamble and ~12us drain
barrier; img_attn rel err ~3.8e-3, ques_attn bit-exact.

Device layouts (per core, P=128):
  imgt [pair, p, k, b2, s] = img[2*pair+b2, s, 128*k+p]   bf16  (feat rhs)
  w2xp [p, k, h]           = w2x[128*k+p, h]              bf16  (feat lhsT)
  w2hp [p, c]              = w2h[128*c+p]                 bf16
  gt   [p, c, b]           = g[b, 128*c+p]                f32   (tanh bias)
  oattn[b, p, n]           = img_attn[b, 128*n+p]         f32   (output)
"""

import numpy as np
import ml_dtypes

BF16 = ml_dtypes.bfloat16

B, S, I, Q, H = 512, 196, 2048, 1024, 512
NCORES = 8
BPC = B // NCORES          # 64 batches per core
P = 128
KI = I // P                # 16 contraction tiles over I
CH = H // P                # 4 chunks over H
NI = I // P                # 16 output column tiles

_CACHE = {}


def build_bass(npairs, fp8=False):
    """Build the per-core Bass program processing 2*npairs batches."""
    import concourse.tile as tile
    from concourse import bacc, mybir
    from concourse.bass import ts

    f32 = mybir.dt.float32
    bf16 = mybir.dt.bfloat16
    fp8e4 = mybir.dt.float8e4
    Act = mybir.ActivationFunctionType
    nb = 2 * npairs

    nc = bacc.Bacc("TRN2", target_bir_lowering=False, debug=False)

    # the last LP pairs finish their weighted sum on TensorE/ScalarE (which
    # are idle by then) instead of VectorE, trimming the end-of-kernel DVE
    # tail; they need a small natural-layout copy of those batches
    LP = 2 if npairs > 4 else 0

    imgt = nc.dram_tensor("imgt", [npairs, P, KI, 2 * S], bf16, kind="ExternalInput").ap()
    w2xp = nc.dram_tensor("w2xp", [P, KI, H], bf16, kind="ExternalInput").ap()
    w2hp = nc.dram_tensor("w2hp", [P, CH], bf16, kind="ExternalInput").ap()
    gt = nc.dram_tensor("gt", [P, CH, nb], f32, kind="ExternalInput").ap()
    oattn = nc.dram_tensor("oattn", [nb, P, NI], f32, kind="ExternalOutput").ap()
    if LP:
        natl = nc.dram_tensor("natl", [2 * LP, S, I], bf16, kind="ExternalInput").ap()
        oattn_l = nc.dram_tensor("oattn_l", [2 * LP, I], f32, kind="ExternalOutput").ap()
    if fp8:
        imgt8 = nc.dram_tensor(
            "imgt8", [npairs, P, KI, 2 * S], fp8e4, kind="ExternalInput"
        ).ap()
        w2xp8 = nc.dram_tensor("w2xp8", [P, KI, H], fp8e4, kind="ExternalInput").ap()

    with tile.TileContext(nc) as tc:
        with (
            tc.tile_pool(name="const", bufs=1) as const,
            tc.tile_pool(name="sb", bufs=5) as sb,
            tc.tile_pool(name="sbs", bufs=3) as sbs,
            tc.tile_pool(name="sbo", bufs=1) as sbo,
            tc.tile_pool(name="psf", bufs=1, space="PSUM") as psf,
            tc.tile_pool(name="pss", bufs=4, space="PSUM") as pss,
        ):
            if fp8:
                w2x_sb = const.tile([P, KI, H], fp8e4)
                w2x_src = w2xp8
            else:
                w2x_sb = const.tile([P, KI, H], bf16)
                w2x_src = w2xp
            # split and interleave the startup-critical loads so the first
            # feat matmuls can begin as soon as their k-slices land
            imgt0_t = sb.tile([P, KI, 2 * S], bf16, tag="imgt")
            w2h_sb = const.tile([P, CH], bf16)
            gt_sb = const.tile([P, CH, nb], f32)
            # pair-0 img rides the scalar HWDGE queue, in parallel with the
            # weights on the sync queue (ScalarE has no other work yet)
            for k2 in range(0, KI, 2):
                nc.sync.dma_start(w2x_sb[:, k2 : k2 + 2], w2x_src[:, k2 : k2 + 2])
                nc.scalar.dma_start(imgt0_t[:, k2 : k2 + 2], imgt[0, :, k2 : k2 + 2])
                if k2 == 0:
                    nc.sync.dma_start(w2h_sb[:], w2hp[:])
                    nc.sync.dma_start(gt_sb[:], gt[:])
            if LP:
                natl_sb = []
                for li in range(2 * LP):
                    na_l = const.tile([P, I], bf16, tag=f"natla{li}")
                    nb_l = const.tile([P, I], bf16, tag=f"natlb{li}")
                    natl_sb.append((na_l, nb_l))
                id11 = const.tile([1, 1], bf16)
                nc.vector.memset(id11[:], 1.0)
            state = {}

            def stage_a(pair):
                if pair == 0:
                    imgt_t = imgt0_t
                else:
                    imgt_t = sb.tile([P, KI, 2 * S], bf16, tag="imgt")
                    nc.sync.dma_start(imgt_t[:], imgt[pair])
                if LP and pair == npairs - LP - 2:
                    # prefetch the natural-layout tail batches (low priority:
                    # emitted only now, two pairs before they are consumed)
                    for li in range(2 * LP):
                        na_l, nb_l = natl_sb[li]
                        nc.sync.dma_start(na_l[:], natl[li, 0:P, :])
                        nc.sync.dma_start(nb_l[0 : S - P, :], natl[li, P:S, :])
                if fp8:
                    imgt8_t = sb.tile([P, KI, 2 * S], fp8e4, tag="imgt8")
                    nc.sync.dma_start(imgt8_t[:], imgt8[pair])
                    feat_rhs = imgt8_t
                else:
                    feat_rhs = imgt_t

                # featT: (H-chunk on partitions, 2*S free), accumulate over I.
                # One PSUM bank per H-chunk so the next pair's chunk-c matmuls
                # only wait on this pair's chunk-c tanh.  fp8 uses DoubleRow
                # (two I-tiles per matmul); w2x is pre-scaled by 32 on host to
                # stay out of e4m3 subnormals and the tanh scale undoes it.
                fpcs = []
                for c in range(CH):
                    fps = psf.tile([P, 512], f32, tag=f"feat{c}")
                    if fp8:
                        for k in range(0, KI, 2):
                            nc.tensor.matmul(
                                fps[:, 0 : 2 * S],
                                w2x_sb[:, k : k + 2, ts(c, P)],
                                feat_rhs[:, k : k + 2, :],
                                start=(k == 0),
                                stop=(k == KI - 2),
                                perf_mode=mybir.MatmulPerfMode.DoubleRow,
                            )
                    else:
                        for k in range(KI):
                            nc.tensor.matmul(
                                fps[:, 0 : 2 * S],
                                w2x_sb[:, k, ts(c, P)],
                                feat_rhs[:, k],
                                start=(k == 0),
                                stop=(k == KI - 1),
                            )
                    fpcs.append(fps)

                # h = tanh(featT + g[b])  (g is a per-partition bias)
                h_t = sbs.tile([P, CH, 2, S], bf16, tag="ht")
                for c in range(CH):
                    for b2 in range(2):
                        bidx = 2 * pair + b2
                        nc.scalar.activation(
                            h_t[:, c, b2],
                            fpcs[c][:, b2 * S : (b2 + 1) * S],
                            Act.Tanh,
                            bias=gt_sb[:, c, bidx : bidx + 1],
                            scale=(1.0 / 32.0) if fp8 else 1.0,
                        )
                state[pair] = (imgt_t, h_t)

            def stage_b(pair):
                imgt_t, h_t = state.pop(pair)
                # score (1, 2*S) = w2h.T @ h for both batches at once
                sc_ps = pss.tile([1, 2 * S], f32, tag="sm")
                for c in range(CH):
                    nc.tensor.matmul(
                        sc_ps[:, :],
                        w2h_sb[:, c : c + 1],
                        h_t[:, c],
                        start=(c == 0),
                        stop=(c == CH - 1),
                    )

                # e = exp(score) (scores are O(1): no max-subtraction) with the
                # softmax denominator accumulated for free
                e_row = sbs.tile([1, 2, S], bf16, tag="er")
                sums = sbs.tile([1, 2], f32, tag="sums")
                rc = sbs.tile([1, 2], f32, tag="rc")
                for b2 in range(2):
                    nc.scalar.activation(
                        e_row[:, b2],
                        sc_ps[:, b2 * S : (b2 + 1) * S],
                        Act.Exp,
                        accum_out=sums[:, b2 : b2 + 1],
                    )
                    nc.vector.reciprocal(rc[:, b2 : b2 + 1], sums[:, b2 : b2 + 1])

                SH = S // 2
                tail_pe = LP and pair >= npairs - LP
                for b2 in range(2):
                    bidx = 2 * pair + b2

                    if tail_pe:
                        # transpose e to (S on partitions, 1), weighted sum as
                        # (M=1, N=512) matmuls over the natural-layout copy,
                        # normalization riding the PSUM->SBUF copy on ScalarE
                        li = 2 * (pair - (npairs - LP)) + b2
                        na_l, nb_l = natl_sb[li]
                        et_a = pss.tile([P, 1], bf16, tag="sm")
                        nc.tensor.transpose(et_a[:, :], e_row[:, b2, 0:P], id11[:, :])
                        et_b = pss.tile([P, 1], bf16, tag="sm")
                        nc.tensor.transpose(et_b[0 : S - P, :], e_row[:, b2, P:S], id11[:, :])
                        eta = sbs.tile([P, 1], bf16, tag="eta")
                        nc.vector.tensor_copy(eta[:, :], et_a[:, :])
                        etb = sbs.tile([P, 1], bf16, tag="etb")
                        nc.vector.tensor_copy(etb[0 : S - P, :], et_b[0 : S - P, :])
                        orow = sbo.tile([1, I], f32, tag=f"or{b2}")
                        for n in range(4):
                            o_ps = pss.tile([1, 512], f32, tag="sm")
                            nc.tensor.matmul(
                                o_ps[:, :], eta[:, :], na_l[:, ts(n, 512)],
                                start=True, stop=False,
                            )
                            nc.tensor.matmul(
                                o_ps[:, :], etb[0 : S - P, :], nb_l[0 : S - P, ts(n, 512)],
                                start=False, stop=True,
                            )
                            nc.scalar.mul(orow[:, ts(n, 512)], o_ps[:, :], rc[:, b2 : b2 + 1])
                        nc.sync.dma_start(oattn_l[li : li + 1, :], orow[:, :])
                        continue

                    # a = e / sum(e), then replicate the row to all 128
                    # partitions on the GpSimd engine
                    a_row = sbs.tile([1, S], bf16, tag=f"ar{b2}")
                    nc.vector.tensor_scalar_mul(
                        a_row[:, :], e_row[:, b2], rc[:, b2 : b2 + 1]
                    )
                    a_bc = sbs.tile([P, 1, S], bf16, tag=f"abc{b2}")
                    nc.gpsimd.partition_broadcast(a_bc[:, 0, :], a_row[:, :])

                    # img_attn[b, 128n+p] = sum_s imgT[p, n, s] * a[s]:
                    # bf16 multiply, fold S in half with one add, then a
                    # free-dim reduce -- reusing the transposed feat tiles
                    scratch = sbs.tile([P, KI, S], bf16, tag=f"sc{b2}")
                    nc.vector.tensor_tensor(
                        scratch[:],
                        imgt_t[:, :, b2 * S : (b2 + 1) * S],
                        a_bc[:].to_broadcast([P, KI, S]),
                        op=mybir.AluOpType.mult,
                    )
                    fold = sbs.tile([P, KI, SH], bf16, tag=f"fo{b2}")
                    nc.vector.tensor_tensor(
                        fold[:],
                        scratch[:, :, 0:SH],
                        scratch[:, :, SH:S],
                        op=mybir.AluOpType.add,
                    )
                    obt = sbs.tile([P, NI], f32, tag=f"obt{b2}")
                    nc.vector.tensor_reduce(
                        obt[:, :], fold[:], axis=mybir.AxisListType.X,
                        op=mybir.AluOpType.add,
                    )
                    nc.sync.dma_start(oattn[bidx], obt[:, :])

            # software pipeline: emit pair p+1's loads/feat/tanh before pair
            # p's score/softmax/weighted-sum, so the static PE instruction
            # order never waits on a just-produced tanh
            stage_a(0)
            for pair in range(1, npairs):
                stage_a(pair)
                stage_b(pair - 1)
            stage_b(npairs - 1)

    nc.compile()
    return nc


def _bf16(x):
    return np.asarray(x, dtype=np.float32).astype(BF16)


def _fp8np():
    from concourse import mybir

    return mybir.dt.np(mybir.dt.float8e4)


def prep_core_inputs(img_core, g_core, fp8=False):
    """Host-side layout prep for one core's 64 batches."""
    nb = img_core.shape[0]
    npairs = nb // 2
    img_bf = _bf16(img_core)                                   # (nb, S, I)
    # [pair, p, k, b2, s] = img[2*pair+b2, s, 128*k+p]
    imgt = np.ascontiguousarray(
        img_bf.reshape(npairs, 2, S, KI, P).transpose(0, 4, 3, 1, 2)
    ).reshape(npairs, P, KI, 2 * S)
    # [p, c, b] = g[b, 128*c+p]
    gt = np.ascontiguousarray(
        np.asarray(g_core, dtype=np.float32).reshape(nb, CH, P).transpose(2, 1, 0)
    )
    m = {"imgt": imgt, "gt": gt}
    LP = 2 if npairs > 4 else 0
    if LP:
        m["natl"] = np.ascontiguousarray(img_bf[-2 * LP :])
    if fp8:
        m["imgt8"] = imgt.astype(_fp8np())
    return m


def prep_weight_inputs(w2x, w2h, fp8=False):
    w2xp = np.ascontiguousarray(_bf16(w2x).reshape(KI, P, H).transpose(1, 0, 2))
    w2hp = np.ascontiguousarray(_bf16(w2h).reshape(CH, P).T)
    m = {"w2xp": w2xp, "w2hp": w2hp}
    if fp8:
        m["w2xp8"] = np.ascontiguousarray(
            (np.asarray(w2x, np.float32) * 32.0).reshape(KI, P, H).transpose(1, 0, 2)
        ).astype(_fp8np())
    return m


def run_device(img_feat, g, wm, trace=False, fp8=False):
    """Run the 8-core SPMD kernel; returns (img_attn (B, I) f32, results obj)."""
    from concourse.bass_utils import run_bass_kernel_spmd

    key = (BPC // 2, fp8)
    if key not in _CACHE:
        _CACHE[key] = build_bass(BPC // 2, fp8=fp8)
    nc = _CACHE[key]

    in_maps = []
    for c in range(NCORES):
        sl = slice(c * BPC, (c + 1) * BPC)
        m = prep_core_inputs(img_feat[sl], g[sl], fp8=fp8)
        m.update(wm)
        in_maps.append(m)
    res = run_bass_kernel_spmd(nc, in_maps, list(range(NCORES)), trace=trace)
    parts = []
    for r in res.results:
        core_out = r["oattn"].transpose(0, 2, 1).reshape(BPC, I)
        if "oattn_l" in r:
            nl = r["oattn_l"].shape[0]
            core_out = np.concatenate([core_out[: BPC - nl], r["oattn_l"]], axis=0)
        parts.append(core_out)
    return np.concatenate(parts, axis=0), res


def kernel(
    ques_feat,
    img_feat,
    w1x,
    b1x,
    w1h,
    b1h,
    w2x,
    b2x,
    w2g,
    b2g,
    w2h,
    b2h,
    w3x,
    b3x,
    w3g,
    b3g,
    w3h,
    b3h,
    trace=False,
    fp8=False,
):
    ques_feat = np.asarray(ques_feat, dtype=np.float32)
    img_feat = np.asarray(img_feat)

    # attn1/attn3 are exact no-ops (softmax over a length-1 axis == 1.0)
    ques_attn = ques_feat.copy()

    # tiny guidance GEMM on host (b2h shifts scores uniformly -> softmax-invariant)
    g = ques_feat @ np.asarray(w2g, dtype=np.float32)
    g = g + np.asarray(b2g, dtype=np.float32) + np.asarray(b2x, dtype=np.float32)

    wm = prep_weight_inputs(w2x, w2h, fp8=fp8)
    img_attn, _ = run_device(img_feat, g, wm, trace=trace, fp8=fp8)
    return (ques_attn, img_attn)


if __name__ == "__main__":
    rng = np.random.default_rng(0)
    ins = {
        "ques_feat": rng.standard_normal((B, Q), dtype=np.float32),
        "img_feat": rng.standard_normal((B, S, I), dtype=np.float32),
        "w2x": rng.standard_normal((I, H), dtype=np.float32) * (I ** -0.5),
        "b2x": np.zeros(H, np.float32),
        "w2g": rng.standard_normal((Q, H), dtype=np.float32) * (Q ** -0.5),
        "b2g": np.zeros(H, np.float32),
        "w2h": rng.standard_normal(H, dtype=np.float32) * (H ** -0.5),
        "b2h": np.float32(0.0),
    }
    zeros = {k: np.float32(0.0) for k in
             ["w1x", "b1x", "w1h", "b1h", "w3x", "b3x", "w3g", "b3g", "w3h", "b3h"]}
    out = kernel(**ins, **zeros)
    print(out[0].shape, out[1].shape)
